# revision 1
# baseline (speedup 1.0000x reference)
"""Trainium2 Bass kernel for nn_MinLSTMCell (B=8, T=4096, D=1024, H=1024).

Self-contained: hardcodes shapes/sharding. Data-parallel over batch B across
8 NeuronCores (one batch element per core), as suggested by the sharding hint.
"""


import numpy as np

import concourse.mybir as mybir
import concourse.tile as tile
from concourse import bacc

B, T, D, H = 8, 4096, 1024, 1024
TB = 512            # t-block (psum free dim)
NTB = T // TB       # 8
NHT = H // 128      # 8 h-tiles
NDK = D // 128      # 8 d-chunks
F32 = mybir.dt.float32
F32R = mybir.dt.float32r
AF = mybir.ActivationFunctionType
OP = mybir.AluOpType


def build_kernel():
    nc = bacc.Bacc()
    xt = nc.dram_tensor("xt", [D, T], F32, kind="ExternalInput")  # x transposed
    wdr = {
        g: nc.dram_tensor(f"w{g}", [D, H], F32, kind="ExternalInput")
        for g in "fih"
    }
    nbf = nc.dram_tensor("nbf", [128, NHT], F32, kind="ExternalInput")   # -bf
    hbi = nc.dram_tensor("hbi", [128, NHT], F32, kind="ExternalInput")   # bi/2
    hbh = nc.dram_tensor("hbh", [128, NHT], F32, kind="ExternalInput")   # bh/2
    b2h = nc.dram_tensor("b2h", [128, NHT], F32, kind="ExternalInput")   # 2*bh
    g4 = nc.dram_tensor("g4", [128, NHT], F32, kind="ExternalInput")     # 4*g0
    out = nc.dram_tensor("out", [H, T], F32, kind="ExternalOutput")

    with tile.TileContext(nc) as tc:
        with (
            tc.tile_pool(name="singles", bufs=1) as singles,
            tc.tile_pool(name="xtp", bufs=18) as xt_p,
            tc.tile_pool(name="pz", bufs=6, space="PSUM") as pz,
            tc.tile_pool(name="ew", bufs=3) as ew,
            tc.tile_pool(name="scan", bufs=9) as scan_p,
            tc.tile_pool(name="outp", bufs=4) as out_p,
        ):
            def emit_xload(tb):
                t0 = tb * TB
                tiles = []
                for k in range(NDK):
                    xk = xt_p.tile([128, TB], F32R, tag="xT")
                    nc.sync.dma_start(
                        xk[:],
                        xt[k * 128:(k + 1) * 128, t0:t0 + TB].bitcast(F32R),
                    )
                    tiles.append(xk)
                return tiles

            # x for tb0 loads before the weights
            xT_cur = emit_xload(0)

            # resident weights (scalar queue): per (gate, d-chunk) [128, H]
            w_sb = {}
            for g in "fih":
                for k in range(NDK):
                    t = singles.tile([128, H], F32R, tag=f"W{g}{k}")
                    eng = nc.scalar if k % 2 == 0 else nc.sync
                    eng.dma_start(
                        t[:], wdr[g][k * 128:(k + 1) * 128, :].bitcast(F32R)
                    )
                    w_sb[(g, k)] = t
            nbf_t = singles.tile([128, NHT], F32, tag="nbf")
            nc.sync.dma_start(nbf_t[:], nbf[:])
            hbi_t = singles.tile([128, NHT], F32, tag="hbi")
            nc.sync.dma_start(hbi_t[:], hbi[:])
            hbh_t = singles.tile([128, NHT], F32, tag="hbh")
            nc.sync.dma_start(hbh_t[:], hbh[:])
            b2h_t = singles.tile([128, NHT], F32, tag="b2h")
            nc.sync.dma_start(b2h_t[:], b2h[:])
            g4_t = singles.tile([128, NHT], F32, tag="g4")
            nc.sync.dma_start(g4_t[:], g4[:])

            s_prev = [None] * NHT
            for tb in range(NTB):
                t0 = tb * TB
                xT = xT_cur
                for ht in range(NHT):
                    hs = slice(ht * 128, (ht + 1) * 128)
                    z = {}
                    for g in "fih":
                        zt = pz.tile([128, TB], F32, tag="z")
                        for k in range(NDK):
                            nc.tensor.matmul(
                                zt[:],
                                w_sb[(g, k)][:, hs],
                                xT[k][:],
                                start=(k == 0),
                                stop=(k == NDK - 1),
                            )
                        z[g] = zt
                    # prefetch next block's xT
                    if tb + 1 < NTB and ht == 0:
                        xT_cur = emit_xload(tb + 1)
                    # ---- ACT phase (single table set: exp+tanh+copy+identity)
                    ef = ew.tile([128, TB], F32, tag="ef")
                    nc.scalar.activation(
                        ef[:], z["f"][:], AF.Exp,
                        bias=nbf_t[:, ht:ht + 1], scale=-1.0,
                    )
                    ti_ = ew.tile([128, TB], F32, tag="ti")
                    nc.scalar.activation(
                        ti_[:], z["i"][:], AF.Tanh,
                        bias=hbi_t[:, ht:ht + 1], scale=0.5,
                    )
                    th_ = ew.tile([128, TB], F32, tag="th")
                    nc.scalar.activation(
                        th_[:], z["h"][:], AF.Tanh,
                        bias=hbh_t[:, ht:ht + 1], scale=0.5,
                    )
                    # tip = ti + 1 (in place)
                    nc.scalar.activation(ti_[:], ti_[:], AF.Copy, bias=1.0)
                    # M = 2*zh + 2*bh
                    m_ = ew.tile([128, TB], F32, tag="m")
                    nc.scalar.activation(
                        m_[:], z["h"][:], AF.Identity,
                        bias=b2h_t[:, ht:ht + 1], scale=2.0,
                    )
                    # ---- DVE phase
                    nc.vector.tensor_tensor(m_[:], m_[:], th_[:], op=OP.max)
                    u = ew.tile([128, TB], F32, tag="u")
                    nc.vector.scalar_tensor_tensor(
                        u[:], ef[:], 1.0, ti_[:], op0=OP.add, op1=OP.mult
                    )
                    # w = (m+1)*u  (in place into m_)
                    nc.vector.scalar_tensor_tensor(
                        m_[:], m_[:], 1.0, u[:], op0=OP.add, op1=OP.mult
                    )
                    s_t = scan_p.tile([128, TB], F32, tag="S")
                    init = (
                        g4_t[:, ht:ht + 1] if tb == 0
                        else s_prev[ht][:, TB - 1:TB]
                    )
                    nc.vector.tensor_tensor_scan(
                        s_t[:], m_[:], m_[:], initial=init,
                        op0=OP.add, op1=OP.bypass,
                    )
                    s_prev[ht] = s_t
                    # dd = 2u+4 (in place), then fq = 1/dd (in place)
                    nc.scalar.activation(u[:], u[:], AF.Copy, bias=4.0, scale=2.0)
                    nc.vector.reciprocal_approx_fast(u[:], u[:])
                    o = out_p.tile([128, TB], F32, tag="o")
                    nc.vector.tensor_mul(o[:], u[:], s_t[:])
                    nc.sync.dma_start(out[hs, t0:t0 + TB], o[:])
    nc.finalize()
    return nc


_NC_CACHE = None


def get_nc():
    global _NC_CACHE
    if _NC_CACHE is None:
        _NC_CACHE = build_kernel()
    return _NC_CACHE


def kernel(x_t, h_prev, Wf, bf, Wi, bi, Wh, bh, _run_opts=None):
    from concourse.bass_utils import run_bass_kernel_spmd

    x_t = np.asarray(x_t, dtype=np.float32)
    h_prev = np.asarray(h_prev, dtype=np.float32)
    Wf = np.ascontiguousarray(np.asarray(Wf, dtype=np.float32))
    Wi = np.ascontiguousarray(np.asarray(Wi, dtype=np.float32))
    Wh = np.ascontiguousarray(np.asarray(Wh, dtype=np.float32))
    bf = np.asarray(bf, dtype=np.float32)
    bi = np.asarray(bi, dtype=np.float32)
    bh = np.asarray(bh, dtype=np.float32)

    nc = get_nc()

    g0 = np.maximum(h_prev + 0.5, 1.0 / (1.0 + np.exp(-h_prev))).astype(np.float32)
    nbf = np.ascontiguousarray((-bf).reshape(NHT, 128).T)
    hbi = np.ascontiguousarray((0.5 * bi).reshape(NHT, 128).T)
    hbh = np.ascontiguousarray((0.5 * bh).reshape(NHT, 128).T)
    b2h = np.ascontiguousarray((2.0 * bh).reshape(NHT, 128).T)

    in_maps = []
    for b in range(B):
        g4 = np.ascontiguousarray((4.0 * g0[b]).reshape(NHT, 128).T)
        in_maps.append({
            "xt": np.ascontiguousarray(x_t[b].T),
            "wf": Wf, "wi": Wi, "wh": Wh,
            "nbf": nbf, "hbi": hbi, "hbh": hbh, "b2h": b2h,
            "g4": g4,
        })

    opts = _run_opts or {}
    res = run_bass_kernel_spmd(nc, in_maps, core_ids=list(range(B)), **opts)

    out = np.empty((B, T + 1, H), dtype=np.float32)
    for b in range(B):
        out[b, 0, :] = g0[b]
        out[b, 1:, :] = res.results[b]["out"].T
    if _run_opts is not None:
        return out, res
    return out



# revision 34
# speedup vs baseline: 1.1095x; 1.1095x over previous
"""Trainium2 Bass kernel for nn_MinLSTMCell (B=8, T=4096, D=1024, H=1024).

Self-contained: hardcodes shapes/sharding. Data-parallel over batch B across
8 NeuronCores (one batch element per core).

Math (verified against the reference):
  zf = x@Wf + bf, zi = x@Wi + bi, zh = x@Wh + bh
  u_h = exp(softplus(-zf) - softplus(-zi)) = (1 + e^{-zf}) * sigmoid(zi)
  g   = max(zh + 0.5, sigmoid(zh))         # = exp(log_g(zh))
  S_t = g0 + sum_{s<=t} u_h,s * g_s        # plain cumsum (a_star is not
                                           #  a running sum in the source)
  out[t] = S_t / (1 + u_h,t)               # f_t = 1/(1+u_h,t)
  out[0] = g0 = max(h0+0.5, sigmoid(h0))
Scaled form used on-chip (only exp/tanh/copy/identity act tables needed):
  zh2 = x@(2*Wh)  (weights pre-doubled)
  q1 = zh2 + 2bh + 1
  ef = e^{-zf-bf}; ti1 = 1 + tanh((zi+bi)/2) = 2*sigmoid(zi+bi)
  u  = (1+ef)*ti1 = 2*u_h
  th = tanh((q1-1)/4) = tanh((zh+bh)/2)
  m1 = max(th + 1, q1) = 2g
  w  = m1*u = 4*u_h*g;  S = 4*g0 + cumsum(w);  out = S / (2u+4)

All matmuls bf16 (the PE is ldweights-rate-bound, so fp8 DoubleRow gains
nothing); elementwise fp32 split across ACT/DVE/Pool with r/o software-
pipelined one tile behind to break cross-engine in-order queue cycles.
"""


import numpy as np
import ml_dtypes

import concourse.mybir as mybir
import concourse.tile as tile
from concourse import bacc

B, T, D, H = 8, 4096, 1024, 1024
TB = 512            # t-block (psum free dim)
NTB = T // TB       # 8
NHT = H // 128      # 8 h-tiles of 128
NDK = D // 128      # 8 d-chunks
F32 = mybir.dt.float32
BF16 = mybir.dt.bfloat16
AF = mybir.ActivationFunctionType
OP = mybir.AluOpType

NP_BF16 = ml_dtypes.bfloat16


def build_kernel():
    nc = bacc.Bacc()
    xb = nc.dram_tensor("xb", [D, T], BF16, kind="ExternalInput")
    wf = nc.dram_tensor("wf", [D, H], BF16, kind="ExternalInput")
    wi = nc.dram_tensor("wi", [D, H], BF16, kind="ExternalInput")
    wh = nc.dram_tensor("wh", [D, H], BF16, kind="ExternalInput")  # 2*Wh
    nbf = nc.dram_tensor("nbf", [128, NHT], F32, kind="ExternalInput")  # -bf
    hbi = nc.dram_tensor("hbi", [128, NHT], F32, kind="ExternalInput")  # bi/2
    b2h = nc.dram_tensor("b2h", [128, NHT], F32, kind="ExternalInput")  # 2bh+1
    g4v = nc.dram_tensor("g4v", [128, NHT], F32, kind="ExternalInput")  # 4*g0
    out = nc.dram_tensor("out", [H, T], F32, kind="ExternalOutput")

    with tile.TileContext(nc) as tc:
        with (
            tc.tile_pool(name="singles", bufs=1) as singles,
            tc.tile_pool(name="xbp", bufs=18) as xb_p,
            tc.tile_pool(name="pz", bufs=8, space="PSUM") as pz,
            tc.tile_pool(name="ew", bufs=3) as ew,
            tc.tile_pool(name="dr", bufs=3) as dr_p,
            tc.tile_pool(name="scan", bufs=9) as scan_p,
            tc.tile_pool(name="outp", bufs=4) as out_p,
        ):
            def emit_xload(tb):
                t0 = tb * TB
                tiles_b = []
                for k in range(NDK):
                    xk = xb_p.tile([128, TB], BF16, tag="xB")
                    nc.sync.dma_start(xk[:], xb[k * 128:(k + 1) * 128, t0:t0 + TB])
                    tiles_b.append(xk)
                return tiles_b

            # x for tb0 loads before the weights
            x_cur = emit_xload(0)

            # resident weights: spread DMAs across four engine queues
            engs = [nc.sync, nc.scalar, nc.gpsimd]
            wf_sb, wi_sb, wh_sb = [], [], []
            for gi, (wd, lst) in enumerate(
                [(wf, wf_sb), (wi, wi_sb), (wh, wh_sb)]
            ):
                for k in range(NDK):
                    t = singles.tile([128, H], BF16, tag=f"W{gi}{k}")
                    engs[(gi * NDK + k) % 3].dma_start(
                        t[:], wd[k * 128:(k + 1) * 128, :]
                    )
                    lst.append(t)

            def vload(name, dram):
                t = singles.tile([128, NHT], F32, tag=name)
                nc.sync.dma_start(t[:], dram[:])
                return t

            nbf_t = vload("nbf", nbf)
            hbi_t = vload("hbi", hbi)
            b2h_t = vload("b2h", b2h)
            g4v_t = vload("g4v", g4v)
            c25_t = singles.tile([128, 1], F32, tag="c25")
            nc.vector.memset(c25_t[:], -0.25)

            s_prev = [None] * NHT
            pending = None  # (d_tile, s_tile, hs, t0) of previous tile
            for tb in range(NTB):
                t0 = tb * TB
                xB = x_cur
                for ht in range(NHT):
                    hs = slice(ht * 128, (ht + 1) * 128)
                    # ---- matmuls. zh first: it feeds the longest
                    # elementwise chain (q1 -> th -> m1 -> w -> scan).
                    zh = pz.tile([128, TB], F32, tag="z")
                    for k in range(NDK):
                        nc.tensor.matmul(
                            zh[:], wh_sb[k][:, hs], xB[k][:],
                            start=(k == 0), stop=(k == NDK - 1),
                        )
                    zf = pz.tile([128, TB], F32, tag="z")
                    for k in range(NDK):
                        nc.tensor.matmul(
                            zf[:], wf_sb[k][:, hs], xB[k][:],
                            start=(k == 0), stop=(k == NDK - 1),
                        )
                    zi = pz.tile([128, TB], F32, tag="z")
                    for k in range(NDK):
                        nc.tensor.matmul(
                            zi[:], wi_sb[k][:, hs], xB[k][:],
                            start=(k == 0), stop=(k == NDK - 1),
                        )
                    # prefetch next block's x
                    if tb + 1 < NTB and ht == 0:
                        x_cur = emit_xload(tb + 1)
                    # ---- ACT: q1 = zh2 + 2bh + 1
                    q1 = ew.tile([128, TB], F32, tag="q1")
                    nc.scalar.activation(
                        q1[:], zh[:], AF.Identity,
                        bias=b2h_t[:, ht:ht + 1],
                    )
                    # th = tanh((zh+bh)/2) = tanh((q1-1)/4)
                    th = ew.tile([128, TB], F32, tag="th")
                    nc.scalar.activation(
                        th[:], q1[:], AF.Tanh, bias=c25_t[:, 0:1], scale=0.25,
                    )
                    ef = ew.tile([128, TB], F32, tag="ef")
                    nc.scalar.activation(
                        ef[:], zf[:], AF.Exp,
                        bias=nbf_t[:, ht:ht + 1], scale=-1.0,
                    )
                    ti = ew.tile([128, TB], F32, tag="ti")
                    nc.scalar.activation(
                        ti[:], zi[:], AF.Tanh,
                        bias=hbi_t[:, ht:ht + 1], scale=0.5,
                    )
                    # ti1 = ti + 1  (= 2*sigmoid(zi+bi))
                    ti1 = ew.tile([128, TB], F32, tag="ti1")
                    nc.scalar.activation(ti1[:], ti[:], AF.Copy, bias=1.0)
                    # ---- DVE: m1 = max(th + 1, q1)  (= 2g)
                    m1 = ew.tile([128, TB], F32, tag="m1")
                    nc.vector.scalar_tensor_tensor(
                        m1[:], th[:], 1.0, q1[:], op0=OP.add, op1=OP.max,
                    )
                    # u = (ef + 1) * ti1   (= 2*u_h)
                    u = ew.tile([128, TB], F32, tag="u")
                    nc.vector.scalar_tensor_tensor(
                        u[:], ef[:], 1.0, ti1[:], op0=OP.add, op1=OP.mult,
                    )
                    # d = 2u + 4
                    d = dr_p.tile([128, TB], F32, tag="d")
                    nc.vector.tensor_scalar(
                        d[:], u[:], 2.0, 4.0, op0=OP.mult, op1=OP.add,
                    )
                    # ---- GPSIMD: w = m1 * u
                    w = ew.tile([128, TB], F32, tag="w")
                    nc.gpsimd.tensor_mul(w[:], m1[:], u[:])
                    # ---- DVE: scan S = cumsum(w) + init
                    s_t = scan_p.tile([128, TB], F32, tag="S")
                    init = (
                        g4v_t[:, ht:ht + 1] if tb == 0
                        else s_prev[ht][:, TB - 1:TB]
                    )
                    nc.vector.tensor_tensor_scan(
                        s_t[:], w[:], w[:], initial=init,
                        op0=OP.add, op1=OP.bypass,
                    )
                    s_prev[ht] = s_t
                    # ---- software-pipelined by one tile to break the
                    # in-order cross-engine queue cycle: emit r/o/store for
                    # the PREVIOUS tile here.
                    if pending is not None:
                        pd, ps, phs, pt0 = pending
                        pr = dr_p.tile([128, TB], F32, tag="r")
                        nc.vector.reciprocal_approx_fast(pr[:], pd[:])
                        po = out_p.tile([128, TB], F32, tag="o")
                        nc.gpsimd.tensor_mul(po[:], pr[:], ps[:])
                        nc.sync.dma_start(out[phs, pt0:pt0 + TB], po[:])
                    pending = (d, s_t, hs, t0)
            # drain the last tile
            pd, ps, phs, pt0 = pending
            pr = dr_p.tile([128, TB], F32, tag="r")
            nc.vector.reciprocal_approx_fast(pr[:], pd[:])
            po = out_p.tile([128, TB], F32, tag="o")
            nc.gpsimd.tensor_mul(po[:], pr[:], ps[:])
            nc.sync.dma_start(out[phs, pt0:pt0 + TB], po[:])
    nc.finalize()
    return nc


_NC_CACHE = None


def get_nc():
    global _NC_CACHE
    if _NC_CACHE is None:
        _NC_CACHE = build_kernel()
    return _NC_CACHE


def prep_in_maps(x_t, h_prev, Wf, bf, Wi, bi, Wh, bh):
    x_t = np.asarray(x_t, dtype=np.float32)
    h_prev = np.asarray(h_prev, dtype=np.float32)
    Wf = np.asarray(Wf, dtype=np.float32)
    Wi = np.asarray(Wi, dtype=np.float32)
    Wh = np.asarray(Wh, dtype=np.float32)
    bf = np.asarray(bf, dtype=np.float32)
    bi = np.asarray(bi, dtype=np.float32)
    bh = np.asarray(bh, dtype=np.float32)

    g0 = np.maximum(h_prev + 0.5, 1.0 / (1.0 + np.exp(-h_prev))).astype(np.float32)

    wf_b = np.ascontiguousarray(Wf.astype(NP_BF16))
    wi_b = np.ascontiguousarray(Wi.astype(NP_BF16))
    wh_b = np.ascontiguousarray((2.0 * Wh).astype(NP_BF16))

    nbf = np.ascontiguousarray((-bf).reshape(NHT, 128).T)
    hbi = np.ascontiguousarray((0.5 * bi).reshape(NHT, 128).T)
    b2h = np.ascontiguousarray((2.0 * bh + 1.0).reshape(NHT, 128).T)

    in_maps = []
    for b in range(B):
        xT = np.ascontiguousarray(x_t[b].T)                       # [D, T] f32
        xb_ = np.ascontiguousarray(xT.astype(NP_BF16))
        g4v = np.ascontiguousarray((4.0 * g0[b]).reshape(NHT, 128).T)
        in_maps.append({
            "xb": xb_,
            "wf": wf_b, "wi": wi_b, "wh": wh_b,
            "nbf": nbf, "hbi": hbi, "b2h": b2h,
            "g4v": g4v,
        })
    return in_maps, g0


def kernel(x_t, h_prev, Wf, bf, Wi, bi, Wh, bh, _run_opts=None):
    from concourse.bass_utils import run_bass_kernel_spmd

    in_maps, g0 = prep_in_maps(x_t, h_prev, Wf, bf, Wi, bi, Wh, bh)
    nc = get_nc()

    opts = _run_opts or {}
    res = run_bass_kernel_spmd(nc, in_maps, core_ids=list(range(B)), **opts)

    out = np.empty((B, T + 1, H), dtype=np.float32)
    for b in range(B):
        out[b, 0, :] = g0[b]
        out[b, 1:, :] = res.results[b]["out"].T
    if _run_opts is not None:
        return out, res
    return out


# revision 41
# speedup vs baseline: 1.1098x; 1.0003x over previous
"""Trainium2 Bass kernel for nn_MinLSTMCell (B=8, T=4096, D=1024, H=1024).

Self-contained: hardcodes shapes/sharding. Data-parallel over batch B across
8 NeuronCores (one batch element per core).

Math (verified against the reference):
  zf = x@Wf + bf, zi = x@Wi + bi, zh = x@Wh + bh
  u_h = exp(softplus(-zf) - softplus(-zi)) = (1 + e^{-zf}) * sigmoid(zi)
  g   = max(zh + 0.5, sigmoid(zh))         # = exp(log_g(zh))
  S_t = g0 + sum_{s<=t} u_h,s * g_s        # plain cumsum (a_star is not
                                           #  a running sum in the source)
  out[t] = S_t / (1 + u_h,t)               # f_t = 1/(1+u_h,t)
  out[0] = g0 = max(h0+0.5, sigmoid(h0))
Scaled form used on-chip (only exp/tanh/copy/identity act tables needed):
  zh2 = x@(2*Wh)  (weights pre-doubled)
  q1 = zh2 + 2bh + 1
  ef = e^{-zf-bf}; ti1 = 1 + tanh((zi+bi)/2) = 2*sigmoid(zi+bi)
  u  = (1+ef)*ti1 = 2*u_h
  th = tanh((q1-1)/4) = tanh((zh+bh)/2)
  m1 = max(th + 1, q1) = 2g
  w  = m1*u = 4*u_h*g;  S = 4*g0 + cumsum(w);  out = S / (2u+4)

All matmuls bf16 (the PE is ldweights-rate-bound, so fp8 DoubleRow gains
nothing); elementwise fp32 split across ACT/DVE/Pool with r/o software-
pipelined one tile behind to break cross-engine in-order queue cycles.
"""


import numpy as np
import ml_dtypes

import concourse.mybir as mybir
import concourse.tile as tile
from concourse import bacc

B, T, D, H = 8, 4096, 1024, 1024
TB = 512            # t-block (psum free dim)
NTB = T // TB       # 8
NHT = H // 128      # 8 h-tiles of 128
NDK = D // 128      # 8 d-chunks
F32 = mybir.dt.float32
BF16 = mybir.dt.bfloat16
AF = mybir.ActivationFunctionType
OP = mybir.AluOpType

NP_BF16 = ml_dtypes.bfloat16


def build_kernel():
    nc = bacc.Bacc()
    xb = nc.dram_tensor("xb", [D, T], BF16, kind="ExternalInput")
    wf = nc.dram_tensor("wf", [D, H], BF16, kind="ExternalInput")
    wi = nc.dram_tensor("wi", [D, H], BF16, kind="ExternalInput")
    wh = nc.dram_tensor("wh", [D, H], BF16, kind="ExternalInput")  # 2*Wh
    nbf = nc.dram_tensor("nbf", [128, NHT], F32, kind="ExternalInput")  # -bf
    hbi = nc.dram_tensor("hbi", [128, NHT], F32, kind="ExternalInput")  # bi/2
    b2h = nc.dram_tensor("b2h", [128, NHT], F32, kind="ExternalInput")  # 2bh+1
    g4v = nc.dram_tensor("g4v", [128, NHT], F32, kind="ExternalInput")  # 4*g0
    out = nc.dram_tensor("out", [H, T], F32, kind="ExternalOutput")

    with tile.TileContext(nc) as tc:
        with (
            tc.tile_pool(name="singles", bufs=1) as singles,
            tc.tile_pool(name="xbp", bufs=18) as xb_p,
            tc.tile_pool(name="pz", bufs=8, space="PSUM") as pz,
            tc.tile_pool(name="ew", bufs=3) as ew,
            tc.tile_pool(name="dr", bufs=3) as dr_p,
            tc.tile_pool(name="scan", bufs=9) as scan_p,
            tc.tile_pool(name="outp", bufs=4) as out_p,
        ):
            def emit_xload(tb, eng=None):
                t0 = tb * TB
                tiles_b = []
                for k in range(NDK):
                    xk = xb_p.tile([128, TB], BF16, tag="xB")
                    (eng or nc.sync).dma_start(
                        xk[:], xb[k * 128:(k + 1) * 128, t0:t0 + TB])
                    tiles_b.append(xk)
                return tiles_b

            # x for tb0 first (needed by every matmul), then weights with
            # wh first (the zh matmuls run first in each tile), striped
            # across all three DMA-capable queues.
            x_cur = emit_xload(0)
            engs = [nc.sync, nc.scalar, nc.gpsimd]
            wf_sb, wi_sb, wh_sb = [], [], []
            n = 0
            for gi, (wd, lst) in enumerate(
                [(wh, wh_sb), (wf, wf_sb), (wi, wi_sb)]
            ):
                for k in range(NDK):
                    t = singles.tile([128, H], BF16, tag=f"W{gi}{k}")
                    engs[n % 3].dma_start(t[:], wd[k * 128:(k + 1) * 128, :])
                    n += 1
                    lst.append(t)

            def vload(name, dram):
                t = singles.tile([128, NHT], F32, tag=name)
                nc.sync.dma_start(t[:], dram[:])
                return t

            nbf_t = vload("nbf", nbf)
            hbi_t = vload("hbi", hbi)
            b2h_t = vload("b2h", b2h)
            g4v_t = vload("g4v", g4v)
            c25_t = singles.tile([128, 1], F32, tag="c25")
            nc.vector.memset(c25_t[:], -0.25)

            s_prev = [None] * NHT
            pending = None  # (d_tile, s_tile, hs, t0) of previous tile
            for tb in range(NTB):
                t0 = tb * TB
                xB = x_cur
                for ht in range(NHT):
                    hs = slice(ht * 128, (ht + 1) * 128)
                    # ---- matmuls. zh first: it feeds the longest
                    # elementwise chain (q1 -> th -> m1 -> w -> scan).
                    zh = pz.tile([128, TB], F32, tag="z")
                    for k in range(NDK):
                        nc.tensor.matmul(
                            zh[:], wh_sb[k][:, hs], xB[k][:],
                            start=(k == 0), stop=(k == NDK - 1),
                        )
                    zf = pz.tile([128, TB], F32, tag="z")
                    for k in range(NDK):
                        nc.tensor.matmul(
                            zf[:], wf_sb[k][:, hs], xB[k][:],
                            start=(k == 0), stop=(k == NDK - 1),
                        )
                    zi = pz.tile([128, TB], F32, tag="z")
                    for k in range(NDK):
                        nc.tensor.matmul(
                            zi[:], wi_sb[k][:, hs], xB[k][:],
                            start=(k == 0), stop=(k == NDK - 1),
                        )
                    # prefetch next block's x
                    if tb + 1 < NTB and ht == 0:
                        x_cur = emit_xload(tb + 1)
                    # ---- ACT: q1 = zh2 + 2bh + 1
                    q1 = ew.tile([128, TB], F32, tag="q1")
                    nc.scalar.activation(
                        q1[:], zh[:], AF.Identity,
                        bias=b2h_t[:, ht:ht + 1],
                    )
                    # th = tanh((zh+bh)/2) = tanh((q1-1)/4)
                    th = ew.tile([128, TB], F32, tag="th")
                    nc.scalar.activation(
                        th[:], q1[:], AF.Tanh, bias=c25_t[:, 0:1], scale=0.25,
                    )
                    ef = ew.tile([128, TB], F32, tag="ef")
                    nc.scalar.activation(
                        ef[:], zf[:], AF.Exp,
                        bias=nbf_t[:, ht:ht + 1], scale=-1.0,
                    )
                    ti = ew.tile([128, TB], F32, tag="ti")
                    nc.scalar.activation(
                        ti[:], zi[:], AF.Tanh,
                        bias=hbi_t[:, ht:ht + 1], scale=0.5,
                    )
                    # ti1 = ti + 1  (= 2*sigmoid(zi+bi))
                    ti1 = ew.tile([128, TB], F32, tag="ti1")
                    nc.scalar.activation(ti1[:], ti[:], AF.Copy, bias=1.0)
                    # ---- DVE: m1 = max(th + 1, q1)  (= 2g)
                    m1 = ew.tile([128, TB], F32, tag="m1")
                    nc.vector.scalar_tensor_tensor(
                        m1[:], th[:], 1.0, q1[:], op0=OP.add, op1=OP.max,
                    )
                    # u = (ef + 1) * ti1   (= 2*u_h)
                    u = ew.tile([128, TB], F32, tag="u")
                    nc.vector.scalar_tensor_tensor(
                        u[:], ef[:], 1.0, ti1[:], op0=OP.add, op1=OP.mult,
                    )
                    # d = 2u + 4
                    d = dr_p.tile([128, TB], F32, tag="d")
                    nc.vector.tensor_scalar(
                        d[:], u[:], 2.0, 4.0, op0=OP.mult, op1=OP.add,
                    )
                    # ---- GPSIMD: w = m1 * u
                    w = ew.tile([128, TB], F32, tag="w")
                    nc.gpsimd.tensor_mul(w[:], m1[:], u[:])
                    # ---- DVE: scan S = cumsum(w) + init
                    s_t = scan_p.tile([128, TB], F32, tag="S")
                    init = (
                        g4v_t[:, ht:ht + 1] if tb == 0
                        else s_prev[ht][:, TB - 1:TB]
                    )
                    nc.vector.tensor_tensor_scan(
                        s_t[:], w[:], w[:], initial=init,
                        op0=OP.add, op1=OP.bypass,
                    )
                    s_prev[ht] = s_t
                    # ---- software-pipelined by one tile to break the
                    # in-order cross-engine queue cycle: emit r/o/store for
                    # the PREVIOUS tile here.
                    if pending is not None:
                        pd, ps, phs, pt0 = pending
                        pr = dr_p.tile([128, TB], F32, tag="r")
                        nc.vector.reciprocal_approx_fast(pr[:], pd[:])
                        po = out_p.tile([128, TB], F32, tag="o")
                        nc.gpsimd.tensor_mul(po[:], pr[:], ps[:])
                        nc.sync.dma_start(out[phs, pt0:pt0 + TB], po[:])
                    pending = (d, s_t, hs, t0)
            # drain the last tile
            pd, ps, phs, pt0 = pending
            pr = dr_p.tile([128, TB], F32, tag="r")
            nc.vector.reciprocal_approx_fast(pr[:], pd[:])
            po = out_p.tile([128, TB], F32, tag="o")
            nc.gpsimd.tensor_mul(po[:], pr[:], ps[:])
            nc.sync.dma_start(out[phs, pt0:pt0 + TB], po[:])
    nc.finalize()
    return nc


_NC_CACHE = None


def get_nc():
    global _NC_CACHE
    if _NC_CACHE is None:
        _NC_CACHE = build_kernel()
    return _NC_CACHE


def prep_in_maps(x_t, h_prev, Wf, bf, Wi, bi, Wh, bh):
    x_t = np.asarray(x_t, dtype=np.float32)
    h_prev = np.asarray(h_prev, dtype=np.float32)
    Wf = np.asarray(Wf, dtype=np.float32)
    Wi = np.asarray(Wi, dtype=np.float32)
    Wh = np.asarray(Wh, dtype=np.float32)
    bf = np.asarray(bf, dtype=np.float32)
    bi = np.asarray(bi, dtype=np.float32)
    bh = np.asarray(bh, dtype=np.float32)

    g0 = np.maximum(h_prev + 0.5, 1.0 / (1.0 + np.exp(-h_prev))).astype(np.float32)

    wf_b = np.ascontiguousarray(Wf.astype(NP_BF16))
    wi_b = np.ascontiguousarray(Wi.astype(NP_BF16))
    wh_b = np.ascontiguousarray((2.0 * Wh).astype(NP_BF16))

    nbf = np.ascontiguousarray((-bf).reshape(NHT, 128).T)
    hbi = np.ascontiguousarray((0.5 * bi).reshape(NHT, 128).T)
    b2h = np.ascontiguousarray((2.0 * bh + 1.0).reshape(NHT, 128).T)

    in_maps = []
    for b in range(B):
        xT = np.ascontiguousarray(x_t[b].T)                       # [D, T] f32
        xb_ = np.ascontiguousarray(xT.astype(NP_BF16))
        g4v = np.ascontiguousarray((4.0 * g0[b]).reshape(NHT, 128).T)
        in_maps.append({
            "xb": xb_,
            "wf": wf_b, "wi": wi_b, "wh": wh_b,
            "nbf": nbf, "hbi": hbi, "b2h": b2h,
            "g4v": g4v,
        })
    return in_maps, g0


def kernel(x_t, h_prev, Wf, bf, Wi, bi, Wh, bh, _run_opts=None):
    from concourse.bass_utils import run_bass_kernel_spmd

    in_maps, g0 = prep_in_maps(x_t, h_prev, Wf, bf, Wi, bi, Wh, bh)
    nc = get_nc()

    opts = _run_opts or {}
    res = run_bass_kernel_spmd(nc, in_maps, core_ids=list(range(B)), **opts)

    out = np.empty((B, T + 1, H), dtype=np.float32)
    for b in range(B):
        out[b, 0, :] = g0[b]
        out[b, 1:, :] = res.results[b]["out"].T
    if _run_opts is not None:
        return out, res
    return out
